# revision 2
# baseline (speedup 1.0000x reference)
"""Causal full attention with learnable (L,L) additive bias, on 8 trn2 cores.

Problem: B=4, L=2048, H=8, E=64.
  scores = einsum("blhe,bshe->bhls", q, k) + causal_mask[None,None]
  scores = where(attn_mask, -inf, scores)
  A = softmax(scale * scores, -1);  out = einsum("bhls,bshd->blhd", A, v)

Sharding: core c gets head c (all 4 batches) -> 4 independent (b,h)
attention problems per core, no cross-core communication.

Device algorithm (per core, per pair p=(b, h=core)), "inj" path:
  S^T[k,q] = I^T @ biasT + K_block^T-as-lhsT @ (scale*Q)^T
             (bias = scale*causal_mask with masked -> -30000, pre-injected
              into the PSUM accumulator by an identity matmul so exp(S+b)
              needs no separate elementwise add/multiply; k on partitions so
              A^T lands in exactly the layout the AV matmul wants)
  R = exp(S^T)                 fp16        (ScalarE, PSUM->SBUF; masked -> 0)
  acc[e,q] += V_aug[kblk]^T @ R            (V_aug has a ones column -> row 64 of
                                            acc accumulates the softmax denom)
Host epilogue: out = (acc[:64]/acc[64]).T per pair.
The per-group dependency chain is PE -> ACT -> PE (2 cross-engine hops); the
earlier expb-multiply path (VARIANT="base") had a third DVE hop and measured
~11% slower on hardware (latency-bound kernel: all engines have idle slack).

Block skip-list is derived from the actual attn_mask/causal_mask content, so
the kernel is correct for any mask; for the causal mask it skips ~half the
blocks.
"""

import math

import numpy as np

import concourse.bass as bass
import concourse.mybir as mybir
import concourse.tile as tile
from concourse import bacc
from concourse.bass_utils import run_bass_kernel_spmd

B, L, H, E = 4, 2048, 8, 64
NCORES = 8
PAIRS = B  # (b, h=core) pairs per core
SCALE = 1.0 / math.sqrt(E)

KB = 128          # k-block (PSUM partition dim of S^T)
NKB = L // KB     # 16
QC = 512          # q-chunk (matmul free dim / PSUM bank width)
NQC = L // QC     # 4
JG = 2            # k-blocks fused per exp/mul instruction (FD = JG*QC)
NG = NKB // JG    # 8 groups
VW = 66           # padded V_aug block width (64 + ones + pad for 4B alignment)

DT = mybir.dt.float16
NPDT = np.float16

# test harness hooks
TRACE = False
LAST = {}
REPS = 1  # bench-only: repeat the whole compute inside the program (For_i)
VARIANT = "injro"  # graded default: bias-inject + adjacent-I matmul order.
# Other values are bench-only probes/variants: base (expb-multiply via DVE),
# act2x|dve2x|pe2x (engine-doubling ablations), half|dbl, ilv|duo|duo8,
# inj8|injst (deeper buffers), injduo, empty.

_cache = {}


def _plan(expbT):
    """Per group g=(2g, 2g+1): inclusive qc range that contains any nonzero
    expb column, or None if the whole group is masked out."""
    nz = np.zeros((NKB, NQC), dtype=bool)
    for j in range(NKB):
        blk = expbT[j * KB : (j + 1) * KB]
        for qc in range(NQC):
            nz[j, qc] = np.any(blk[:, qc * QC : (qc + 1) * QC])
    ranges = []
    for g in range(NG):
        v = nz[2 * g] | nz[2 * g + 1]
        idx = np.flatnonzero(v)
        ranges.append((int(idx[0]), int(idx[-1])) if len(idx) else None)
    return tuple(ranges)


def _plan_trim(expbT):
    """Per (j, qc): leading all-zero (fully masked) column count, rounded
    down to a multiple of 128 — those output columns get no contribution
    from this k-block, so its inject/K/AV matmuls can skip them. QC means
    the whole (j, qc) half-block is dead."""
    trim = []
    for j in range(NKB):
        row = [0] * NQC
        blk = expbT[j * KB : (j + 1) * KB]
        for qc in range(NQC):
            cols = np.flatnonzero(
                np.any(blk[:, qc * QC : (qc + 1) * QC], axis=0))
            row[qc] = (int(cols[0]) // 128) * 128 if len(cols) else QC
        trim.append(tuple(row))
    return tuple(trim)


def _expb_cols(ranges):
    return sum((qce - qcs + 1) * JG * QC for r in ranges if r is not None
               for qcs, qce in [r])


def _build(ranges, smin=None):
    """Build the SPMD Bass program for one core (identical on all cores)."""
    expb_cols = _expb_cols(ranges)
    nc = bacc.Bacc("TRN2", target_bir_lowering=False, debug=False)

    qt = [nc.dram_tensor(f"qt{p}", [64, L], DT, kind="ExternalInput").ap()
          for p in range(PAIRS)]
    kt = [nc.dram_tensor(f"kt{p}", [64, L], DT, kind="ExternalInput").ap()
          for p in range(PAIRS)]
    inj = VARIANT in ("inj", "inj8", "injst", "injduo", "injtrim", "injro", "injroduo")
    vaug = nc.dram_tensor("vaug", [128, PAIRS * NKB * VW], DT,
                          kind="ExternalInput").ap()
    if inj:
        biast = nc.dram_tensor("biast", [128, expb_cols], DT,
                               kind="ExternalInput").ap()
        ident = nc.dram_tensor("ident", [128, 128], DT,
                               kind="ExternalInput").ap()
    else:
        expb = nc.dram_tensor("expb", [128, expb_cols], DT,
                              kind="ExternalInput").ap()
    ot = nc.dram_tensor("ot", [PAIRS, E + 1, L], mybir.dt.float32,
                        kind="ExternalOutput").ap()

    # group g's packed-expb column offset
    goff = {}
    off = 0
    for g, r in enumerate(ranges):
        if r is None:
            continue
        goff[g] = off
        off += (r[1] - r[0] + 1) * JG * QC

    f32 = mybir.dt.float32
    with tile.TileContext(nc) as tc:
        with (
            tc.tile_pool(name="const", bufs=1) as const_pool,
            tc.tile_pool(name="work", bufs=3) as work_pool,
            tc.tile_pool(name="ostage", bufs=2) as out_pool,
            tc.tile_pool(name="spsum", bufs=3, space="PSUM") as s_pool,
            tc.tile_pool(name="apsum", bufs=2, space="PSUM") as acc_pool,
        ):
            qt_sb = [const_pool.tile([64, L], DT, tag=f"qt{p}", name=f"qt{p}_sb")
                     for p in range(PAIRS)]
            kt_sb = [const_pool.tile([64, L], DT, tag=f"kt{p}", name=f"kt{p}_sb")
                     for p in range(PAIRS)]
            vaug_sb = const_pool.tile([128, PAIRS * NKB * VW], DT, tag="vaug")
            bias_sb = const_pool.tile([128, expb_cols], DT, tag="expb")
            expb_sb = bias_sb

            nc.sync.dma_start(qt_sb[0][:], qt[0])
            nc.sync.dma_start(kt_sb[0][:], kt[0])
            if inj:
                ident_sb = const_pool.tile([128, 128], DT, tag="ident")
                nc.sync.dma_start(ident_sb[:], ident)
            nc.sync.dma_start(vaug_sb[:], vaug)
            # split the big bias load so compute can start on early chunks
            bias_src = biast if inj else expb
            nsplit = 8
            step = -(-expb_cols // nsplit)
            step += step % 2  # keep 4B alignment
            for a in range(0, expb_cols, step):
                b = min(a + step, expb_cols)
                nc.sync.dma_start(bias_sb[:, a:b], bias_src[:, a:b])
            for p in range(1, PAIRS):
                nc.sync.dma_start(qt_sb[p][:], qt[p])
                nc.sync.dma_start(kt_sb[p][:], kt[p])

            def compute_ilv():
                # qc outer, group middle, pair inner: 4 independent
                # per-pair dependency chains in flight at all times.
                for qc in range(NQC):
                    gs = [g for g in range(NG)
                          if ranges[g] is not None
                          and ranges[g][0] <= qc <= ranges[g][1]]
                    if not gs:
                        continue
                    accs = [acc_pool.tile([E + 1, QC], f32, tag=f"acc{p}",
                                          bufs=1, name=f"acc{p}")
                            for p in range(PAIRS)]
                    nmm = JG * len(gs)
                    mms = [0] * PAIRS
                    for g in gs:
                        boff = goff[g] + (qc - ranges[g][0]) * JG * QC
                        for p in range(PAIRS):
                            s_t = s_pool.tile([128, JG * QC], f32, tag="s",
                                              bufs=2, name="s_t")
                            for t in range(JG):
                                j = JG * g + t
                                nc.tensor.matmul(
                                    s_t[:, t * QC : (t + 1) * QC],
                                    kt_sb[p][:, j * KB : (j + 1) * KB],
                                    qt_sb[p][:, qc * QC : (qc + 1) * QC],
                                    start=True, stop=True,
                                )
                            e_t = work_pool.tile([128, JG * QC], DT, tag="exp",
                                                 bufs=6, name="e_t")
                            nc.scalar.activation(
                                e_t[:], s_t[:], mybir.ActivationFunctionType.Exp
                            )
                            r_t = work_pool.tile([128, JG * QC], DT, tag="rhs",
                                                 bufs=6, name="r_t")
                            nc.vector.tensor_mul(
                                r_t[:], e_t[:], expb_sb[:, boff : boff + JG * QC]
                            )
                            for t in range(JG):
                                j = JG * g + t
                                voff = (p * NKB + j) * VW
                                nc.tensor.matmul(
                                    accs[p][:],
                                    vaug_sb[:, voff : voff + E + 1],
                                    r_t[:, t * QC : (t + 1) * QC],
                                    start=(mms[p] == 0),
                                    stop=(mms[p] == nmm - 1),
                                )
                                mms[p] += 1
                    for p in range(PAIRS):
                        st = out_pool.tile([E + 1, QC], f32, tag="st",
                                           name="st")
                        nc.vector.tensor_copy(st[:], accs[p][:])
                        nc.sync.dma_start(ot[p][:, qc * QC : (qc + 1) * QC],
                                          st[:])

            def make_rhs(p, g, qc, boff, sbufs, wb, s_g=0):
                """S = (bias +) K^T Q for group g, then rhs = exp(S)(*expb).
                s_g: leading all-masked columns per half chunk to skip."""
                s_t = s_pool.tile([128, JG * QC], f32, tag="s", bufs=sbufs,
                                  name="s_t")
                ro = VARIANT in ("injro", "injroduo")  # I-injects first: I,I,K0,K1
                order = ([(t, True) for t in range(JG)]
                         + [(t, False) for t in range(JG)]) if (inj and ro)                     else [(t, w) for t in range(JG)
                          for w in ((True, False) if inj else (False,))]
                for t, is_inj in order:
                    j = JG * g + t
                    sl = s_t[:, t * QC + s_g : (t + 1) * QC]
                    if is_inj:
                        nc.tensor.matmul(
                            sl, ident_sb[:],
                            bias_sb[:, boff + t * QC + s_g
                                    : boff + (t + 1) * QC],
                            start=True, stop=False,
                            skip_group_check=ro)
                    else:
                        nc.tensor.matmul(
                            sl,
                            kt_sb[p][:, j * KB : (j + 1) * KB],
                            qt_sb[p][:, qc * QC + s_g : (qc + 1) * QC],
                            start=not inj, stop=True,
                            skip_group_check=ro)
                if inj:
                    r_t = work_pool.tile([128, JG * QC], DT, tag="rhs",
                                         bufs=wb, name="r_t")
                    if s_g:
                        s3 = s_t[:].rearrange(
                            "p (t w) -> p t w", t=JG)[:, :, s_g:]
                        r3 = r_t[:].rearrange(
                            "p (t w) -> p t w", t=JG)[:, :, s_g:]
                        nc.scalar.activation(
                            r3, s3, mybir.ActivationFunctionType.Exp)
                    else:
                        nc.scalar.activation(
                            r_t[:], s_t[:], mybir.ActivationFunctionType.Exp)
                else:
                    e_t = work_pool.tile([128, JG * QC], DT, tag="exp",
                                         bufs=wb, name="e_t")
                    nc.scalar.activation(
                        e_t[:], s_t[:], mybir.ActivationFunctionType.Exp)
                    r_t = work_pool.tile([128, JG * QC], DT, tag="rhs",
                                         bufs=wb, name="r_t")
                    nc.vector.tensor_mul(
                        r_t[:], e_t[:], expb_sb[:, boff : boff + JG * QC])
                return r_t

            def compute_inj():
                # bias pre-injected into PSUM by an identity matmul:
                # chain is PE -> ACT -> PE (DVE only does output copies).
                wb = {"inj8": 8, "injst": 6}.get(VARIANT, 4)
                stb = 4 if VARIANT == "injst" else 2
                for p in range(PAIRS):
                    for qc in range(NQC):
                        gs = [g for g in range(NG)
                              if ranges[g] is not None
                              and ranges[g][0] <= qc <= ranges[g][1]]
                        if not gs:
                            continue
                        acc = acc_pool.tile([E + 1, QC], f32, tag="acc")
                        nmm = JG * len(gs)
                        mm = 0
                        for gi, g in enumerate(gs):
                            boff = goff[g] + (qc - ranges[g][0]) * JG * QC
                            s_g = 0
                            if VARIANT == "injtrim" and smin and gi > 0:
                                s_g = smin[g][qc]
                            r_t = make_rhs(p, g, qc, boff, 3, wb, s_g)
                            for t in range(JG):
                                j = JG * g + t
                                voff = (p * NKB + j) * VW
                                nc.tensor.matmul(
                                    acc[:, s_g:],
                                    vaug_sb[:, voff : voff + E + 1],
                                    r_t[:, t * QC + s_g : (t + 1) * QC],
                                    start=(mm == 0), stop=(mm == nmm - 1))
                                mm += 1
                        st = out_pool.tile([E + 1, QC], f32, tag="st",
                                           bufs=stb, name="st")
                        nc.vector.tensor_copy(st[:], acc[:])
                        nc.sync.dma_start(
                            ot[p][:, qc * QC : (qc + 1) * QC], st[:])

            def compute_duo():
                # two independent pair-chains (A: pairs 0/1, B: pairs 2/3)
                # interleaved; keeps s_pool triple-buffered (2+3*2=8 banks).
                wb = 8 if VARIANT == "duo8" else 3
                for half in range(2):
                    pA, pB = 2 * half, 2 * half + 1
                    for qc in range(NQC):
                        gs = [g for g in range(NG)
                              if ranges[g] is not None
                              and ranges[g][0] <= qc <= ranges[g][1]]
                        if not gs:
                            continue
                        accs = {p: acc_pool.tile([E + 1, QC], f32,
                                                 tag=f"acc{p % 2}", bufs=1,
                                                 name=f"acc{p}")
                                for p in (pA, pB)}
                        nmm = JG * len(gs)
                        mms = {pA: 0, pB: 0}
                        for g in gs:
                            boff = goff[g] + (qc - ranges[g][0]) * JG * QC
                            for p in (pA, pB):
                                r_t = make_rhs(p, g, qc, boff, 3, wb)
                                for t in range(JG):
                                    j = JG * g + t
                                    voff = (p * NKB + j) * VW
                                    nc.tensor.matmul(
                                        accs[p][:],
                                        vaug_sb[:, voff : voff + E + 1],
                                        r_t[:, t * QC : (t + 1) * QC],
                                        start=(mms[p] == 0),
                                        stop=(mms[p] == nmm - 1),
                                    )
                                    mms[p] += 1
                        for p in (pA, pB):
                            st = out_pool.tile([E + 1, QC], f32, tag="st",
                                               bufs=4, name="st")
                            nc.vector.tensor_copy(st[:], accs[p][:])
                            nc.sync.dma_start(
                                ot[p][:, qc * QC : (qc + 1) * QC], st[:])

            def compute():
              if VARIANT in ("inj", "inj8", "injst", "injtrim", "injro"):
                  compute_inj()
                  return
              if VARIANT in ("duo", "duo8", "injduo", "injroduo"):
                  compute_duo()
                  return
              if VARIANT == "empty":
                  st = out_pool.tile([E + 1, QC], f32, tag="st", name="st0")
                  nc.vector.tensor_copy(st[:], expb_sb[: E + 1, :QC])
                  return
              if VARIANT == "ilv":
                  compute_ilv()
                  return
              npairs = 2 if VARIANT == "half" else PAIRS
              for p in range(npairs):
                qts = qt_sb[p]
                kts = kt_sb[p]
                lo, hi = 0, 64
                for qc in range(NQC):
                    gs = [g for g in range(NG)
                          if ranges[g] is not None
                          and ranges[g][0] <= qc <= ranges[g][1]]
                    if not gs:
                        continue
                    acc = acc_pool.tile([E + 1, QC], f32, tag="acc")
                    nmm = JG * len(gs) * (2 if VARIANT == "pe2x" else 1)
                    mm = 0
                    for g in gs:
                        s_t = s_pool.tile([128, JG * QC], f32, tag="s")
                        for t in range(JG):
                            j = JG * g + t
                            for _dup in range(2 if VARIANT == "pe2x" else 1):
                                nc.tensor.matmul(
                                    s_t[:, t * QC : (t + 1) * QC],
                                    kts[lo:hi, j * KB : (j + 1) * KB],
                                    qts[lo:hi, qc * QC : (qc + 1) * QC],
                                    start=True, stop=True,
                                )
                        e_t = work_pool.tile([128, JG * QC], DT, tag="exp")
                        nc.scalar.activation(
                            e_t[:], s_t[:], mybir.ActivationFunctionType.Exp
                        )
                        if VARIANT == "act2x":
                            e_t2 = work_pool.tile([128, JG * QC], DT, tag="exp2")
                            nc.scalar.copy(e_t2[:], e_t[:])
                            e_t = e_t2
                        r_t = work_pool.tile([128, JG * QC], DT, tag="rhs")
                        boff = goff[g] + (qc - ranges[g][0]) * JG * QC
                        nc.vector.tensor_mul(
                            r_t[:], e_t[:], expb_sb[:, boff : boff + JG * QC]
                        )
                        if VARIANT == "dve2x":
                            r_t2 = work_pool.tile([128, JG * QC], DT, tag="rhs2")
                            nc.vector.tensor_mul(
                                r_t2[:], r_t[:], expb_sb[:, boff : boff + JG * QC]
                            )
                            r_t = r_t2
                        for t in range(JG):
                            j = JG * g + t
                            voff = (p * NKB + j) * VW
                            for dup in range(2 if VARIANT == "pe2x" else 1):
                                nc.tensor.matmul(
                                    acc[:],
                                    vaug_sb[:, voff : voff + E + 1],
                                    r_t[:, t * QC : (t + 1) * QC],
                                    start=(mm == 0), stop=(mm == nmm - 1),
                                )
                                mm += 1
                    st = out_pool.tile([E + 1, QC], f32, tag="st")
                    nc.vector.tensor_copy(st[:], acc[:])
                    nc.sync.dma_start(ot[p][:, qc * QC : (qc + 1) * QC], st[:])

            def compute_n():
                compute()
                if VARIANT == "dbl":
                    compute()

            if REPS > 1:
                hints = (mybir.EngineType.PE, mybir.EngineType.Activation,
                         mybir.EngineType.DVE, mybir.EngineType.SP)
                with tc.For_i(0, REPS, 1, hint_engines=hints):
                    compute_n()
            else:
                compute_n()
    nc.compile()
    return nc


MASK_NEG = -30000.0  # additive bias for masked entries (fp16-safe, exp -> 0)


def _pack(matT, ranges):
    """Pack a [k, q]-oriented (L, L) matrix into the group-major SBUF layout:
    per group g, per qc in its range, [j0-chunk | j1-chunk]."""
    chunks = []
    for g, r in enumerate(ranges):
        if r is None:
            continue
        for qc in range(r[0], r[1] + 1):
            for t in range(JG):
                j = JG * g + t
                chunks.append(
                    matT[j * KB : (j + 1) * KB, qc * QC : (qc + 1) * QC])
    return np.ascontiguousarray(np.concatenate(chunks, axis=1))


def prepare(queries, keys, values, attn_mask, causal_mask):
    """Host-side prep: block plan + per-core input maps."""
    queries = np.asarray(queries, dtype=np.float32)
    keys = np.asarray(keys, dtype=np.float32)
    values = np.asarray(values, dtype=np.float32)
    attn_mask = np.asarray(attn_mask).astype(bool).reshape(L, L)
    causal_mask = np.asarray(causal_mask, dtype=np.float32)
    assert queries.shape == (B, L, H, E)

    # exp of the scaled additive bias, 0 where masked; [k, q] orientation
    expbT = np.where(attn_mask, 0.0, np.exp(SCALE * causal_mask)).T
    expbT = np.ascontiguousarray(expbT, dtype=NPDT)
    # raw additive bias with masked -> large negative (inject path)
    biasT = np.where(attn_mask, MASK_NEG, SCALE * causal_mask).T
    biasT = np.ascontiguousarray(biasT, dtype=NPDT)

    ranges = _plan(expbT != 0)
    smin = _plan_smin(expbT != 0, ranges)
    expb_packed = _pack(expbT, ranges)
    bias_packed = _pack(biasT, ranges)
    ident = np.eye(128, dtype=NPDT)

    in_maps = []
    for c in range(NCORES):
        qts, kts = [], []
        va = np.zeros((128, PAIRS * NKB * VW), dtype=NPDT)
        for p in range(PAIRS):
            qts.append((queries[p, :, c, :].T * SCALE).astype(NPDT))
            kts.append(keys[p, :, c, :].T.astype(NPDT))
            vp = values[p, :, c, :].astype(NPDT)  # (L, 64)
            for j in range(NKB):
                col = (p * NKB + j) * VW
                va[:, col : col + E] = vp[j * KB : (j + 1) * KB, :]
                va[:, col + E] = 1.0
        im = {"vaug": va, "expb": expb_packed, "biast": bias_packed,
              "ident": ident}
        for p in range(PAIRS):
            im[f"qt{p}"] = np.ascontiguousarray(qts[p])
            im[f"kt{p}"] = np.ascontiguousarray(kts[p])
        in_maps.append(im)
    return ranges, smin, in_maps


def kernel(queries, keys, values, attn_mask, causal_mask):
    ranges, smin, in_maps = prepare(queries, keys, values, attn_mask,
                                    causal_mask)
    key = (ranges, smin, VARIANT)
    if key not in _cache:
        _cache[key] = _build(ranges, smin)
    nc = _cache[key]

    res = run_bass_kernel_spmd(nc, in_maps, list(range(NCORES)), trace=TRACE)
    LAST["results"] = res

    out = np.empty((B, L, H, E), dtype=np.float32)
    for c in range(NCORES):
        ot = res.results[c]["ot"]  # (PAIRS, 65, L)
        for p in range(PAIRS):
            out[p, :, c, :] = (ot[p, :E, :] / ot[p, E : E + 1, :]).T
    return out



# revision 14
# speedup vs baseline: 1.0697x; 1.0697x over previous
"""Causal full attention with learnable (L,L) additive bias, on 8 trn2 cores.

Problem: B=4, L=2048, H=8, E=64.
  scores = einsum("blhe,bshe->bhls", q, k) + causal_mask[None,None]
  scores = where(attn_mask, -inf, scores)
  A = softmax(scale * scores, -1);  out = einsum("bhls,bshd->blhd", A, v)

Sharding: core c gets head c (all 4 batches) -> 4 independent (b,h)
attention problems per core, no cross-core communication.

Device algorithm (per core, per pair p=(b, h=core)), "inj" path:
  S^T[k,q] = I^T @ biasT + K_block^T-as-lhsT @ (scale*Q)^T
             (bias = scale*causal_mask with masked -> -30000, pre-injected
              into the PSUM accumulator by an identity matmul so exp(S+b)
              needs no separate elementwise add/multiply; k on partitions so
              A^T lands in exactly the layout the AV matmul wants)
  R = exp(S^T)                 fp16        (ScalarE, PSUM->SBUF; masked -> 0)
  acc[e,q] += V_aug[kblk]^T @ R            (V_aug has a ones column -> row 64 of
                                            acc accumulates the softmax denom)
Host epilogue: out = (acc[:64]/acc[64]).T per pair.
The per-group dependency chain is PE -> ACT -> PE (2 cross-engine hops); the
earlier expb-multiply path (VARIANT="base") had a third DVE hop and measured
~11% slower on hardware (latency-bound kernel: all engines have idle slack).

Block skip-list is derived from the actual attn_mask/causal_mask content, so
the kernel is correct for any mask; for the causal mask it skips ~half the
blocks.
"""

import math

import numpy as np

import concourse.bass as bass
import concourse.mybir as mybir
import concourse.tile as tile
from concourse import bacc
from concourse.bass_utils import run_bass_kernel_spmd

B, L, H, E = 4, 2048, 8, 64
NCORES = 8
PAIRS = B  # (b, h=core) pairs per core
SCALE = 1.0 / math.sqrt(E)

KB = 128          # k-block (PSUM partition dim of S^T)
NKB = L // KB     # 16
QC = 512          # q-chunk (matmul free dim / PSUM bank width)
NQC = L // QC     # 4
JG = 2            # k-blocks fused per exp/mul instruction (FD = JG*QC)
NG = NKB // JG    # 8 groups
VW = 66           # padded V_aug block width (64 + ones + pad for 4B alignment)

DT = mybir.dt.float16
NPDT = np.float16

# test harness hooks
TRACE = False
LAST = {}
REPS = 1  # bench-only: repeat the whole compute inside the program (For_i)
# per-qc count of full (untrimmed) blocks whose bias is applied as an
# expb-multiply on DVE instead of a PE inject (mix variant only): balances
# PE against the idle DVE capacity.
MIX_MUL = (0, 1, 2, 2)
VARIANT = "injro"  # graded default: bias-inject + adjacent-I matmul order.
# Other values are bench-only probes/variants: base (expb-multiply via DVE),
# act2x|dve2x|pe2x (engine-doubling ablations), half|dbl, ilv|duo|duo8,
# inj8|injst (deeper buffers), injduo, empty.

_cache = {}


def _plan(expbT):
    """Per group g=(2g, 2g+1): inclusive qc range that contains any nonzero
    expb column, or None if the whole group is masked out."""
    nz = np.zeros((NKB, NQC), dtype=bool)
    for j in range(NKB):
        blk = expbT[j * KB : (j + 1) * KB]
        for qc in range(NQC):
            nz[j, qc] = np.any(blk[:, qc * QC : (qc + 1) * QC])
    ranges = []
    for g in range(NG):
        v = nz[2 * g] | nz[2 * g + 1]
        idx = np.flatnonzero(v)
        ranges.append((int(idx[0]), int(idx[-1])) if len(idx) else None)
    return tuple(ranges)


def _plan_trim(expbT):
    """Per (j, qc): leading all-zero (fully masked) column count, rounded
    down to a multiple of 128 — those output columns get no contribution
    from this k-block, so its inject/K/AV matmuls can skip them. QC means
    the whole (j, qc) half-block is dead."""
    trim = []
    for j in range(NKB):
        row = [0] * NQC
        blk = expbT[j * KB : (j + 1) * KB]
        for qc in range(NQC):
            cols = np.flatnonzero(
                np.any(blk[:, qc * QC : (qc + 1) * QC], axis=0))
            row[qc] = (int(cols[0]) // 128) * 128 if len(cols) else QC
        trim.append(tuple(row))
    return tuple(trim)


def _expb_cols(ranges):
    return sum((qce - qcs + 1) * JG * QC for r in ranges if r is not None
               for qcs, qce in [r])


def _mix_mul_blocks(ranges, trim):
    """(g, qc) blocks whose bias is applied as an expb-multiply on DVE in
    the mix variant. Only full (untrimmed) blocks qualify, spread evenly
    across groups; MIX_MUL[qc] picks how many per q-chunk."""
    out = set()
    for qc in range(NQC):
        cand = [g for g in range(NG)
                if ranges[g] is not None
                and ranges[g][0] <= qc <= ranges[g][1]
                and all(trim[JG * g + t][qc] == 0 for t in range(JG))]
        k = min(MIX_MUL[qc], len(cand))
        if k:
            step = len(cand) / k
            out.update((cand[int(i * step)], qc) for i in range(k))
    return frozenset(out)


def _build(ranges, trim=None):
    """Build the SPMD Bass program for one core (identical on all cores)."""
    expb_cols = _expb_cols(ranges)
    nc = bacc.Bacc("TRN2", target_bir_lowering=False, debug=False)

    qt = [nc.dram_tensor(f"qt{p}", [64, L], DT, kind="ExternalInput").ap()
          for p in range(PAIRS)]
    kt = [nc.dram_tensor(f"kt{p}", [64, L], DT, kind="ExternalInput").ap()
          for p in range(PAIRS)]
    inj = VARIANT in ("inj", "inj8", "injst", "injduo", "injtrim", "injro",
                      "injroduo", "pipe", "pipet", "mix")
    vaug = nc.dram_tensor("vaug", [128, PAIRS * NKB * VW], DT,
                          kind="ExternalInput").ap()
    if inj:
        biast = nc.dram_tensor("biast", [128, expb_cols], DT,
                               kind="ExternalInput").ap()
        ident = nc.dram_tensor("ident", [128, 128], DT,
                               kind="ExternalInput").ap()
    else:
        expb = nc.dram_tensor("expb", [128, expb_cols], DT,
                              kind="ExternalInput").ap()
    ot = nc.dram_tensor("ot", [PAIRS, E + 1, L], mybir.dt.float32,
                        kind="ExternalOutput").ap()

    # group g's packed-expb column offset
    goff = {}
    off = 0
    for g, r in enumerate(ranges):
        if r is None:
            continue
        goff[g] = off
        off += (r[1] - r[0] + 1) * JG * QC

    f32 = mybir.dt.float32
    with tile.TileContext(nc) as tc:
        with (
            tc.tile_pool(name="const", bufs=1) as const_pool,
            tc.tile_pool(name="work", bufs=3) as work_pool,
            tc.tile_pool(name="ostage", bufs=2) as out_pool,
            tc.tile_pool(name="spsum", bufs=3, space="PSUM") as s_pool,
            tc.tile_pool(name="apsum", bufs=2, space="PSUM") as acc_pool,
        ):
            qt_sb = [const_pool.tile([64, L], DT, tag=f"qt{p}", name=f"qt{p}_sb")
                     for p in range(PAIRS)]
            kt_sb = [const_pool.tile([64, L], DT, tag=f"kt{p}", name=f"kt{p}_sb")
                     for p in range(PAIRS)]
            vaug_sb = const_pool.tile([128, PAIRS * NKB * VW], DT, tag="vaug")
            bias_sb = const_pool.tile([128, expb_cols], DT, tag="expb")
            expb_sb = bias_sb

            nc.sync.dma_start(qt_sb[0][:], qt[0])
            nc.sync.dma_start(kt_sb[0][:], kt[0])
            if inj:
                ident_sb = const_pool.tile([128, 128], DT, tag="ident")
                nc.sync.dma_start(ident_sb[:], ident)
            nc.sync.dma_start(vaug_sb[:], vaug)
            # split the big bias load so compute can start on early chunks
            bias_src = biast if inj else expb
            nsplit = 8
            step = -(-expb_cols // nsplit)
            step += step % 2  # keep 4B alignment
            for a in range(0, expb_cols, step):
                b = min(a + step, expb_cols)
                nc.sync.dma_start(bias_sb[:, a:b], bias_src[:, a:b])
            for p in range(1, PAIRS):
                nc.sync.dma_start(qt_sb[p][:], qt[p])
                nc.sync.dma_start(kt_sb[p][:], kt[p])

            def compute_ilv():
                # qc outer, group middle, pair inner: 4 independent
                # per-pair dependency chains in flight at all times.
                for qc in range(NQC):
                    gs = [g for g in range(NG)
                          if ranges[g] is not None
                          and ranges[g][0] <= qc <= ranges[g][1]]
                    if not gs:
                        continue
                    accs = [acc_pool.tile([E + 1, QC], f32, tag=f"acc{p}",
                                          bufs=1, name=f"acc{p}")
                            for p in range(PAIRS)]
                    nmm = JG * len(gs)
                    mms = [0] * PAIRS
                    for g in gs:
                        boff = goff[g] + (qc - ranges[g][0]) * JG * QC
                        for p in range(PAIRS):
                            s_t = s_pool.tile([128, JG * QC], f32, tag="s",
                                              bufs=2, name="s_t")
                            for t in range(JG):
                                j = JG * g + t
                                nc.tensor.matmul(
                                    s_t[:, t * QC : (t + 1) * QC],
                                    kt_sb[p][:, j * KB : (j + 1) * KB],
                                    qt_sb[p][:, qc * QC : (qc + 1) * QC],
                                    start=True, stop=True,
                                )
                            e_t = work_pool.tile([128, JG * QC], DT, tag="exp",
                                                 bufs=6, name="e_t")
                            nc.scalar.activation(
                                e_t[:], s_t[:], mybir.ActivationFunctionType.Exp
                            )
                            r_t = work_pool.tile([128, JG * QC], DT, tag="rhs",
                                                 bufs=6, name="r_t")
                            nc.vector.tensor_mul(
                                r_t[:], e_t[:], expb_sb[:, boff : boff + JG * QC]
                            )
                            for t in range(JG):
                                j = JG * g + t
                                voff = (p * NKB + j) * VW
                                nc.tensor.matmul(
                                    accs[p][:],
                                    vaug_sb[:, voff : voff + E + 1],
                                    r_t[:, t * QC : (t + 1) * QC],
                                    start=(mms[p] == 0),
                                    stop=(mms[p] == nmm - 1),
                                )
                                mms[p] += 1
                    for p in range(PAIRS):
                        st = out_pool.tile([E + 1, QC], f32, tag="st",
                                           name="st")
                        nc.vector.tensor_copy(st[:], accs[p][:])
                        nc.sync.dma_start(ot[p][:, qc * QC : (qc + 1) * QC],
                                          st[:])

            def make_rhs(p, g, qc, boff, sbufs, wb, s_g=0):
                """S = (bias +) K^T Q for group g, then rhs = exp(S)(*expb).
                s_g: leading all-masked columns per half chunk to skip."""
                s_t = s_pool.tile([128, JG * QC], f32, tag="s", bufs=sbufs,
                                  name="s_t")
                ro = VARIANT in ("injro", "injroduo")  # I-injects first: I,I,K0,K1
                order = ([(t, True) for t in range(JG)]
                         + [(t, False) for t in range(JG)]) if (inj and ro)                     else [(t, w) for t in range(JG)
                          for w in ((True, False) if inj else (False,))]
                for t, is_inj in order:
                    j = JG * g + t
                    sl = s_t[:, t * QC + s_g : (t + 1) * QC]
                    if is_inj:
                        nc.tensor.matmul(
                            sl, ident_sb[:],
                            bias_sb[:, boff + t * QC + s_g
                                    : boff + (t + 1) * QC],
                            start=True, stop=False,
                            skip_group_check=ro)
                    else:
                        nc.tensor.matmul(
                            sl,
                            kt_sb[p][:, j * KB : (j + 1) * KB],
                            qt_sb[p][:, qc * QC + s_g : (qc + 1) * QC],
                            start=not inj, stop=True,
                            skip_group_check=ro)
                if inj:
                    r_t = work_pool.tile([128, JG * QC], DT, tag="rhs",
                                         bufs=wb, name="r_t")
                    if s_g:
                        s3 = s_t[:].rearrange(
                            "p (t w) -> p t w", t=JG)[:, :, s_g:]
                        r3 = r_t[:].rearrange(
                            "p (t w) -> p t w", t=JG)[:, :, s_g:]
                        nc.scalar.activation(
                            r3, s3, mybir.ActivationFunctionType.Exp)
                    else:
                        nc.scalar.activation(
                            r_t[:], s_t[:], mybir.ActivationFunctionType.Exp)
                else:
                    e_t = work_pool.tile([128, JG * QC], DT, tag="exp",
                                         bufs=wb, name="e_t")
                    nc.scalar.activation(
                        e_t[:], s_t[:], mybir.ActivationFunctionType.Exp)
                    r_t = work_pool.tile([128, JG * QC], DT, tag="rhs",
                                         bufs=wb, name="r_t")
                    nc.vector.tensor_mul(
                        r_t[:], e_t[:], expb_sb[:, boff : boff + JG * QC])
                return r_t

            def compute_inj():
                # bias pre-injected into PSUM by an identity matmul:
                # chain is PE -> ACT -> PE (DVE only does output copies).
                wb = {"inj8": 8, "injst": 6}.get(VARIANT, 4)
                stb = 4 if VARIANT == "injst" else 2
                for p in range(PAIRS):
                    for qc in range(NQC):
                        gs = [g for g in range(NG)
                              if ranges[g] is not None
                              and ranges[g][0] <= qc <= ranges[g][1]]
                        if not gs:
                            continue
                        acc = acc_pool.tile([E + 1, QC], f32, tag="acc")
                        nmm = JG * len(gs)
                        mm = 0
                        for gi, g in enumerate(gs):
                            boff = goff[g] + (qc - ranges[g][0]) * JG * QC
                            s_g = 0
                            if VARIANT == "injtrim" and trim and gi > 0:
                                s_g = min(trim[JG * g + t][qc]
                                          for t in range(JG))
                                s_g = 0 if s_g >= QC else s_g
                            r_t = make_rhs(p, g, qc, boff, 3, wb, s_g)
                            for t in range(JG):
                                j = JG * g + t
                                voff = (p * NKB + j) * VW
                                nc.tensor.matmul(
                                    acc[:, s_g:],
                                    vaug_sb[:, voff : voff + E + 1],
                                    r_t[:, t * QC + s_g : (t + 1) * QC],
                                    start=(mm == 0), stop=(mm == nmm - 1))
                                mm += 1
                        st = out_pool.tile([E + 1, QC], f32, tag="st",
                                           bufs=stb, name="st")
                        nc.vector.tensor_copy(st[:], acc[:])
                        nc.sync.dma_start(
                            ot[p][:, qc * QC : (qc + 1) * QC], st[:])

            def make_rhs_pipe(p, g, qc, boff, dd, is_mul=False):
                """S-block with per-half leading-col trim dd=(d0, d1).
                Inject covers [dmin:] on both halves so exp reads only
                defined PSUM; K matmuls cover each half's own [d_t:].
                is_mul: apply the bias as exp(S)*expb on DVE instead of a
                PE inject (only used for untrimmed blocks)."""
                dmin = min(dd)
                s_t = s_pool.tile([128, JG * QC], f32, tag="s", bufs=3,
                                  name="s_t")
                if not is_mul:
                    for t in range(JG):
                        nc.tensor.matmul(
                            s_t[:, t * QC + dmin : (t + 1) * QC],
                            ident_sb[:],
                            bias_sb[:, boff + t * QC + dmin
                                    : boff + (t + 1) * QC],
                            start=True, stop=False, skip_group_check=True)
                for t in range(JG):
                    if dd[t] >= QC:
                        continue
                    j = JG * g + t
                    nc.tensor.matmul(
                        s_t[:, t * QC + dd[t] : (t + 1) * QC],
                        kt_sb[p][:, j * KB : (j + 1) * KB],
                        qt_sb[p][:, qc * QC + dd[t] : (qc + 1) * QC],
                        start=is_mul, stop=True, skip_group_check=True)
                r_t = work_pool.tile([128, JG * QC], DT, tag="rhs", bufs=8,
                                     name="r_t")
                if is_mul:
                    e_t = work_pool.tile([128, JG * QC], DT, tag="exp",
                                         bufs=4, name="e_t")
                    nc.scalar.activation(
                        e_t[:], s_t[:], mybir.ActivationFunctionType.Exp)
                    nc.vector.tensor_mul(
                        r_t[:], e_t[:], expb_sb[:, boff : boff + JG * QC])
                elif dmin:
                    s3 = s_t[:].rearrange("p (t w) -> p t w", t=JG)[:, :, dmin:]
                    r3 = r_t[:].rearrange("p (t w) -> p t w", t=JG)[:, :, dmin:]
                    nc.scalar.activation(
                        r3, s3, mybir.ActivationFunctionType.Exp)
                else:
                    nc.scalar.activation(
                        r_t[:], s_t[:], mybir.ActivationFunctionType.Exp)
                return r_t

            def compute_pipe():
                # Two software-pipelined chains (A: pairs 0/2, B: 1/3); each
                # block's AV matmuls are emitted one block AFTER its S-block,
                # with the other chain's work in between, so the in-order PE
                # never waits on ACT's exp.
                dtab = (trim if VARIANT == "pipet" and trim is not None
                        else tuple((0,) * NQC for _ in range(NKB)))

                def blocks_for(plist):
                    blks = []
                    for p in plist:
                        for qc in range(NQC):
                            gs = [g for g in range(NG)
                                  if ranges[g] is not None
                                  and ranges[g][0] <= qc <= ranges[g][1]]
                            nmm = sum(1 for g in gs for t in range(JG)
                                      if dtab[JG * g + t][qc] < QC)
                            for i, g in enumerate(gs):
                                boff = (goff[g]
                                        + (qc - ranges[g][0]) * JG * QC)
                                blks.append(dict(
                                    p=p, qc=qc, g=g, boff=boff,
                                    first=(i == 0), last=(i == len(gs) - 1),
                                    nmm=nmm))
                    return blks

                chains = [blocks_for([0, 2]), blocks_for([1, 3])]
                nb = len(chains[0])
                assert len(chains[1]) == nb and nb >= 2
                rstore = {}
                st_acc = [None, None]
                st_mm = [0, 0]

                def emit_S(c, k):
                    b = chains[c][k]
                    dd = tuple(dtab[JG * b["g"] + t][b["qc"]]
                               for t in range(JG))
                    rstore[(c, k)] = (
                        make_rhs_pipe(b["p"], b["g"], b["qc"], b["boff"], dd),
                        dd)

                def emit_AV(c, k):
                    b = chains[c][k]
                    r_t, dd = rstore.pop((c, k))
                    if st_mm[c] == 0:
                        st_acc[c] = acc_pool.tile([E + 1, QC], f32,
                                                  tag="acc", name="acc")
                    for t in range(JG):
                        if dd[t] >= QC:
                            continue
                        j = JG * b["g"] + t
                        voff = (b["p"] * NKB + j) * VW
                        nc.tensor.matmul(
                            st_acc[c][:, dd[t]:],
                            vaug_sb[:, voff : voff + E + 1],
                            r_t[:, t * QC + dd[t] : (t + 1) * QC],
                            start=(st_mm[c] == 0),
                            stop=(st_mm[c] == b["nmm"] - 1))
                        st_mm[c] += 1
                    if b["last"]:
                        st = out_pool.tile([E + 1, QC], f32, tag="st",
                                           bufs=4, name="st")
                        nc.vector.tensor_copy(st[:], st_acc[c][:])
                        nc.sync.dma_start(
                            ot[b["p"]][:, b["qc"] * QC : (b["qc"] + 1) * QC],
                            st[:])
                        st_mm[c] = 0

                emit_S(0, 0)
                emit_S(1, 0)
                for k in range(1, nb):
                    emit_S(0, k)
                    emit_AV(0, k - 1)
                    emit_S(1, k)
                    emit_AV(1, k - 1)
                emit_AV(0, nb - 1)
                emit_AV(1, nb - 1)

            def compute_duo():
                # two independent pair-chains (A: pairs 0/1, B: pairs 2/3)
                # interleaved; keeps s_pool triple-buffered (2+3*2=8 banks).
                wb = 8 if VARIANT == "duo8" else 3
                for half in range(2):
                    pA, pB = 2 * half, 2 * half + 1
                    for qc in range(NQC):
                        gs = [g for g in range(NG)
                              if ranges[g] is not None
                              and ranges[g][0] <= qc <= ranges[g][1]]
                        if not gs:
                            continue
                        accs = {p: acc_pool.tile([E + 1, QC], f32,
                                                 tag=f"acc{p % 2}", bufs=1,
                                                 name=f"acc{p}")
                                for p in (pA, pB)}
                        nmm = JG * len(gs)
                        mms = {pA: 0, pB: 0}
                        for g in gs:
                            boff = goff[g] + (qc - ranges[g][0]) * JG * QC
                            for p in (pA, pB):
                                r_t = make_rhs(p, g, qc, boff, 3, wb)
                                for t in range(JG):
                                    j = JG * g + t
                                    voff = (p * NKB + j) * VW
                                    nc.tensor.matmul(
                                        accs[p][:],
                                        vaug_sb[:, voff : voff + E + 1],
                                        r_t[:, t * QC : (t + 1) * QC],
                                        start=(mms[p] == 0),
                                        stop=(mms[p] == nmm - 1),
                                    )
                                    mms[p] += 1
                        for p in (pA, pB):
                            st = out_pool.tile([E + 1, QC], f32, tag="st",
                                               bufs=4, name="st")
                            nc.vector.tensor_copy(st[:], accs[p][:])
                            nc.sync.dma_start(
                                ot[p][:, qc * QC : (qc + 1) * QC], st[:])

            def compute():
              if VARIANT in ("pipe", "pipet"):
                  compute_pipe()
                  return
              if VARIANT in ("inj", "inj8", "injst", "injtrim", "injro"):
                  compute_inj()
                  return
              if VARIANT in ("duo", "duo8", "injduo", "injroduo"):
                  compute_duo()
                  return
              if VARIANT == "empty":
                  st = out_pool.tile([E + 1, QC], f32, tag="st", name="st0")
                  nc.vector.tensor_copy(st[:], expb_sb[: E + 1, :QC])
                  return
              if VARIANT == "ilv":
                  compute_ilv()
                  return
              npairs = 2 if VARIANT == "half" else PAIRS
              for p in range(npairs):
                qts = qt_sb[p]
                kts = kt_sb[p]
                lo, hi = 0, 64
                for qc in range(NQC):
                    gs = [g for g in range(NG)
                          if ranges[g] is not None
                          and ranges[g][0] <= qc <= ranges[g][1]]
                    if not gs:
                        continue
                    acc = acc_pool.tile([E + 1, QC], f32, tag="acc")
                    nmm = JG * len(gs) * (2 if VARIANT == "pe2x" else 1)
                    mm = 0
                    for g in gs:
                        s_t = s_pool.tile([128, JG * QC], f32, tag="s")
                        for t in range(JG):
                            j = JG * g + t
                            for _dup in range(2 if VARIANT == "pe2x" else 1):
                                nc.tensor.matmul(
                                    s_t[:, t * QC : (t + 1) * QC],
                                    kts[lo:hi, j * KB : (j + 1) * KB],
                                    qts[lo:hi, qc * QC : (qc + 1) * QC],
                                    start=True, stop=True,
                                )
                        e_t = work_pool.tile([128, JG * QC], DT, tag="exp")
                        nc.scalar.activation(
                            e_t[:], s_t[:], mybir.ActivationFunctionType.Exp
                        )
                        if VARIANT == "act2x":
                            e_t2 = work_pool.tile([128, JG * QC], DT, tag="exp2")
                            nc.scalar.copy(e_t2[:], e_t[:])
                            e_t = e_t2
                        r_t = work_pool.tile([128, JG * QC], DT, tag="rhs")
                        boff = goff[g] + (qc - ranges[g][0]) * JG * QC
                        nc.vector.tensor_mul(
                            r_t[:], e_t[:], expb_sb[:, boff : boff + JG * QC]
                        )
                        if VARIANT == "dve2x":
                            r_t2 = work_pool.tile([128, JG * QC], DT, tag="rhs2")
                            nc.vector.tensor_mul(
                                r_t2[:], r_t[:], expb_sb[:, boff : boff + JG * QC]
                            )
                            r_t = r_t2
                        for t in range(JG):
                            j = JG * g + t
                            voff = (p * NKB + j) * VW
                            for dup in range(2 if VARIANT == "pe2x" else 1):
                                nc.tensor.matmul(
                                    acc[:],
                                    vaug_sb[:, voff : voff + E + 1],
                                    r_t[:, t * QC : (t + 1) * QC],
                                    start=(mm == 0), stop=(mm == nmm - 1),
                                )
                                mm += 1
                    st = out_pool.tile([E + 1, QC], f32, tag="st")
                    nc.vector.tensor_copy(st[:], acc[:])
                    nc.sync.dma_start(ot[p][:, qc * QC : (qc + 1) * QC], st[:])

            def compute_n():
                compute()
                if VARIANT == "dbl":
                    compute()

            if REPS > 1:
                hints = (mybir.EngineType.PE, mybir.EngineType.Activation,
                         mybir.EngineType.DVE, mybir.EngineType.SP)
                with tc.For_i(0, REPS, 1, hint_engines=hints):
                    compute_n()
            else:
                compute_n()
    nc.compile()
    return nc


MASK_NEG = -30000.0  # additive bias for masked entries (fp16-safe, exp -> 0)


def _pack(matT, ranges):
    """Pack a [k, q]-oriented (L, L) matrix into the group-major SBUF layout:
    per group g, per qc in its range, [j0-chunk | j1-chunk]."""
    chunks = []
    for g, r in enumerate(ranges):
        if r is None:
            continue
        for qc in range(r[0], r[1] + 1):
            for t in range(JG):
                j = JG * g + t
                chunks.append(
                    matT[j * KB : (j + 1) * KB, qc * QC : (qc + 1) * QC])
    return np.ascontiguousarray(np.concatenate(chunks, axis=1))


def prepare(queries, keys, values, attn_mask, causal_mask):
    """Host-side prep: block plan + per-core input maps."""
    queries = np.asarray(queries, dtype=np.float32)
    keys = np.asarray(keys, dtype=np.float32)
    values = np.asarray(values, dtype=np.float32)
    attn_mask = np.asarray(attn_mask).astype(bool).reshape(L, L)
    causal_mask = np.asarray(causal_mask, dtype=np.float32)
    assert queries.shape == (B, L, H, E)

    # exp of the scaled additive bias, 0 where masked; [k, q] orientation
    expbT = np.where(attn_mask, 0.0, np.exp(SCALE * causal_mask)).T
    expbT = np.ascontiguousarray(expbT, dtype=NPDT)
    # raw additive bias with masked -> large negative (inject path)
    biasT = np.where(attn_mask, MASK_NEG, SCALE * causal_mask).T
    biasT = np.ascontiguousarray(biasT, dtype=NPDT)

    ranges = _plan(expbT != 0)
    trim = _plan_trim(expbT != 0)
    expb_packed = _pack(expbT, ranges)
    bias_packed = _pack(biasT, ranges)
    ident = np.eye(128, dtype=NPDT)

    in_maps = []
    for c in range(NCORES):
        qts, kts = [], []
        va = np.zeros((128, PAIRS * NKB * VW), dtype=NPDT)
        for p in range(PAIRS):
            qts.append((queries[p, :, c, :].T * SCALE).astype(NPDT))
            kts.append(keys[p, :, c, :].T.astype(NPDT))
            vp = values[p, :, c, :].astype(NPDT)  # (L, 64)
            for j in range(NKB):
                col = (p * NKB + j) * VW
                va[:, col : col + E] = vp[j * KB : (j + 1) * KB, :]
                va[:, col + E] = 1.0
        im = {"vaug": va, "expb": expb_packed, "biast": bias_packed,
              "ident": ident}
        for p in range(PAIRS):
            im[f"qt{p}"] = np.ascontiguousarray(qts[p])
            im[f"kt{p}"] = np.ascontiguousarray(kts[p])
        in_maps.append(im)
    return ranges, trim, in_maps


def kernel(queries, keys, values, attn_mask, causal_mask):
    ranges, trim, in_maps = prepare(queries, keys, values, attn_mask,
                                    causal_mask)
    key = (ranges, trim, VARIANT)
    if key not in _cache:
        _cache[key] = _build(ranges, trim)
    nc = _cache[key]

    res = run_bass_kernel_spmd(nc, in_maps, list(range(NCORES)), trace=TRACE)
    LAST["results"] = res

    out = np.empty((B, L, H, E), dtype=np.float32)
    for c in range(NCORES):
        ot = res.results[c]["ot"]  # (PAIRS, 65, L)
        for p in range(PAIRS):
            out[p, :, c, :] = (ot[p, :E, :] / ot[p, E : E + 1, :]).T
    return out



# revision 21
# speedup vs baseline: 1.6646x; 1.5561x over previous
"""Causal full attention with learnable (L,L) additive bias, on 8 trn2 cores.

Problem: B=4, L=2048, H=8, E=64.
  scores = einsum("blhe,bshe->bhls", q, k) + causal_mask[None,None]
  scores = where(attn_mask, -inf, scores)
  A = softmax(scale * scores, -1);  out = einsum("bhls,bshd->blhd", A, v)

Sharding: core c gets head c (all 4 batches) -> 4 independent (b,h)
attention problems per core, no cross-core communication.

Device algorithm (per core, per pair p=(b, h=core)), "inj" path:
  S^T[k,q] = I^T @ biasT + K_block^T-as-lhsT @ (scale*Q)^T
             (bias = scale*causal_mask with masked -> -30000, pre-injected
              into the PSUM accumulator by an identity matmul so exp(S+b)
              needs no separate elementwise add/multiply; k on partitions so
              A^T lands in exactly the layout the AV matmul wants)
  R = exp(S^T)                 fp16        (ScalarE, PSUM->SBUF; masked -> 0)
  acc[e,q] += V_aug[kblk]^T @ R            (V_aug has a ones column -> row 64 of
                                            acc accumulates the softmax denom)
Host epilogue: out = (acc[:64]/acc[64]).T per pair.
The per-group dependency chain is PE -> ACT -> PE (2 cross-engine hops); the
earlier expb-multiply path (VARIANT="base") had a third DVE hop and measured
~11% slower on hardware (latency-bound kernel: all engines have idle slack).

Block skip-list is derived from the actual attn_mask/causal_mask content, so
the kernel is correct for any mask; for the causal mask it skips ~half the
blocks.
"""

import math

import numpy as np

import concourse.bass as bass
import concourse.mybir as mybir
import concourse.tile as tile
from concourse import bacc
from concourse.bass_utils import run_bass_kernel_spmd

B, L, H, E = 4, 2048, 8, 64
NCORES = 8
PAIRS = B  # (b, h=core) pairs per core
SCALE = 1.0 / math.sqrt(E)

KB = 128          # k-block (PSUM partition dim of S^T)
NKB = L // KB     # 16
QC = 512          # q-chunk (matmul free dim / PSUM bank width)
NQC = L // QC     # 4
JG = 2            # k-blocks fused per exp/mul instruction (FD = JG*QC)
NG = NKB // JG    # 8 groups
VW = 66           # padded V_aug block width (64 + ones + pad for 4B alignment)

DT = mybir.dt.float16
NPDT = np.float16

# test harness hooks
TRACE = False
LAST = {}
REPS = 1  # bench-only: repeat the whole compute inside the program (For_i)
# per-qc count of full (untrimmed) blocks whose bias is applied as an
# expb-multiply on DVE instead of a PE inject (mix variant only): balances
# PE against the idle DVE capacity.
MIX_MUL = (0, 1, 2, 2)
VARIANT = "injro"  # graded default: bias-inject + adjacent-I matmul order.
# Other values are bench-only probes/variants: base (expb-multiply via DVE),
# act2x|dve2x|pe2x (engine-doubling ablations), half|dbl, ilv|duo|duo8,
# inj8|injst (deeper buffers), injduo, empty.

_cache = {}


def _plan(expbT):
    """Per group g=(2g, 2g+1): inclusive qc range that contains any nonzero
    expb column, or None if the whole group is masked out."""
    nz = np.zeros((NKB, NQC), dtype=bool)
    for j in range(NKB):
        blk = expbT[j * KB : (j + 1) * KB]
        for qc in range(NQC):
            nz[j, qc] = np.any(blk[:, qc * QC : (qc + 1) * QC])
    ranges = []
    for g in range(NG):
        v = nz[2 * g] | nz[2 * g + 1]
        idx = np.flatnonzero(v)
        ranges.append((int(idx[0]), int(idx[-1])) if len(idx) else None)
    return tuple(ranges)


def _plan_trim(expbT):
    """Per (j, qc): leading all-zero (fully masked) column count, rounded
    down to a multiple of 128 — those output columns get no contribution
    from this k-block, so its inject/K/AV matmuls can skip them. QC means
    the whole (j, qc) half-block is dead."""
    trim = []
    for j in range(NKB):
        row = [0] * NQC
        blk = expbT[j * KB : (j + 1) * KB]
        for qc in range(NQC):
            cols = np.flatnonzero(
                np.any(blk[:, qc * QC : (qc + 1) * QC], axis=0))
            row[qc] = (int(cols[0]) // 128) * 128 if len(cols) else QC
        trim.append(tuple(row))
    return tuple(trim)


def _expb_cols(ranges):
    return sum((qce - qcs + 1) * JG * QC for r in ranges if r is not None
               for qcs, qce in [r])


def _mix_mul_blocks(ranges, trim):
    """(g, qc) blocks whose bias is applied as an expb-multiply on DVE in
    the mix variant. Only full (untrimmed) blocks qualify, spread evenly
    across groups; MIX_MUL[qc] picks how many per q-chunk."""
    out = set()
    for qc in range(NQC):
        cand = [g for g in range(NG)
                if ranges[g] is not None
                and ranges[g][0] <= qc <= ranges[g][1]
                and all(trim[JG * g + t][qc] == 0 for t in range(JG))]
        k = min(MIX_MUL[qc], len(cand))
        if k:
            step = len(cand) / k
            out.update((cand[int(i * step)], qc) for i in range(k))
    return frozenset(out)


def _build(ranges, trim=None):
    """Build the SPMD Bass program for one core (identical on all cores)."""
    expb_cols = _expb_cols(ranges)
    nc = bacc.Bacc("TRN2", target_bir_lowering=False, debug=False)

    qt = [nc.dram_tensor(f"qt{p}", [64, L], DT, kind="ExternalInput").ap()
          for p in range(PAIRS)]
    kt = [nc.dram_tensor(f"kt{p}", [64, L], DT, kind="ExternalInput").ap()
          for p in range(PAIRS)]
    inj = VARIANT in ("inj", "inj8", "injst", "injduo", "injtrim", "injro",
                      "injroduo", "pipe", "pipet", "mix")
    vaug = nc.dram_tensor("vaug", [128, PAIRS * NKB * VW], DT,
                          kind="ExternalInput").ap()
    if inj:
        biast = nc.dram_tensor("biast", [128, expb_cols], DT,
                               kind="ExternalInput").ap()
        ident = nc.dram_tensor("ident", [128, 128], DT,
                               kind="ExternalInput").ap()
    else:
        expb = nc.dram_tensor("expb", [128, expb_cols], DT,
                              kind="ExternalInput").ap()
    ot = nc.dram_tensor("ot", [PAIRS, E + 1, L], mybir.dt.float32,
                        kind="ExternalOutput").ap()

    # group g's packed-expb column offset
    goff = {}
    off = 0
    for g, r in enumerate(ranges):
        if r is None:
            continue
        goff[g] = off
        off += (r[1] - r[0] + 1) * JG * QC

    f32 = mybir.dt.float32
    with tile.TileContext(nc) as tc:
        with (
            tc.tile_pool(name="const", bufs=1) as const_pool,
            tc.tile_pool(name="work", bufs=3) as work_pool,
            tc.tile_pool(name="ostage", bufs=2) as out_pool,
            tc.tile_pool(name="spsum", bufs=3, space="PSUM") as s_pool,
            tc.tile_pool(name="apsum", bufs=2, space="PSUM") as acc_pool,
        ):
            qt_sb = [const_pool.tile([64, L], DT, tag=f"qt{p}", name=f"qt{p}_sb")
                     for p in range(PAIRS)]
            kt_sb = [const_pool.tile([64, L], DT, tag=f"kt{p}", name=f"kt{p}_sb")
                     for p in range(PAIRS)]
            vaug_sb = const_pool.tile([128, PAIRS * NKB * VW], DT, tag="vaug")
            bias_sb = const_pool.tile([128, expb_cols], DT, tag="expb")
            expb_sb = bias_sb

            nc.sync.dma_start(qt_sb[0][:], qt[0])
            nc.sync.dma_start(kt_sb[0][:], kt[0])
            if inj:
                ident_sb = const_pool.tile([128, 128], DT, tag="ident")
                nc.sync.dma_start(ident_sb[:], ident)
            nc.sync.dma_start(vaug_sb[:], vaug)
            # split the big bias load so compute can start on early chunks
            bias_src = biast if inj else expb
            nsplit = 8
            step = -(-expb_cols // nsplit)
            step += step % 2  # keep 4B alignment
            for a in range(0, expb_cols, step):
                b = min(a + step, expb_cols)
                nc.sync.dma_start(bias_sb[:, a:b], bias_src[:, a:b])
            for p in range(1, PAIRS):
                nc.sync.dma_start(qt_sb[p][:], qt[p])
                nc.sync.dma_start(kt_sb[p][:], kt[p])

            def compute_ilv():
                # qc outer, group middle, pair inner: 4 independent
                # per-pair dependency chains in flight at all times.
                for qc in range(NQC):
                    gs = [g for g in range(NG)
                          if ranges[g] is not None
                          and ranges[g][0] <= qc <= ranges[g][1]]
                    if not gs:
                        continue
                    accs = [acc_pool.tile([E + 1, QC], f32, tag=f"acc{p}",
                                          bufs=1, name=f"acc{p}")
                            for p in range(PAIRS)]
                    nmm = JG * len(gs)
                    mms = [0] * PAIRS
                    for g in gs:
                        boff = goff[g] + (qc - ranges[g][0]) * JG * QC
                        for p in range(PAIRS):
                            s_t = s_pool.tile([128, JG * QC], f32, tag="s",
                                              bufs=2, name="s_t")
                            for t in range(JG):
                                j = JG * g + t
                                nc.tensor.matmul(
                                    s_t[:, t * QC : (t + 1) * QC],
                                    kt_sb[p][:, j * KB : (j + 1) * KB],
                                    qt_sb[p][:, qc * QC : (qc + 1) * QC],
                                    start=True, stop=True,
                                )
                            e_t = work_pool.tile([128, JG * QC], DT, tag="exp",
                                                 bufs=6, name="e_t")
                            nc.scalar.activation(
                                e_t[:], s_t[:], mybir.ActivationFunctionType.Exp
                            )
                            r_t = work_pool.tile([128, JG * QC], DT, tag="rhs",
                                                 bufs=6, name="r_t")
                            nc.vector.tensor_mul(
                                r_t[:], e_t[:], expb_sb[:, boff : boff + JG * QC]
                            )
                            for t in range(JG):
                                j = JG * g + t
                                voff = (p * NKB + j) * VW
                                nc.tensor.matmul(
                                    accs[p][:],
                                    vaug_sb[:, voff : voff + E + 1],
                                    r_t[:, t * QC : (t + 1) * QC],
                                    start=(mms[p] == 0),
                                    stop=(mms[p] == nmm - 1),
                                )
                                mms[p] += 1
                    for p in range(PAIRS):
                        st = out_pool.tile([E + 1, QC], f32, tag="st",
                                           name="st")
                        nc.vector.tensor_copy(st[:], accs[p][:])
                        nc.sync.dma_start(ot[p][:, qc * QC : (qc + 1) * QC],
                                          st[:])

            def make_rhs(p, g, qc, boff, sbufs, wb, s_g=0):
                """S = (bias +) K^T Q for group g, then rhs = exp(S)(*expb).
                s_g: leading all-masked columns per half chunk to skip."""
                s_t = s_pool.tile([128, JG * QC], f32, tag="s", bufs=sbufs,
                                  name="s_t")
                ro = VARIANT in ("injro", "injroduo")  # I-injects first: I,I,K0,K1
                order = ([(t, True) for t in range(JG)]
                         + [(t, False) for t in range(JG)]) if (inj and ro)                     else [(t, w) for t in range(JG)
                          for w in ((True, False) if inj else (False,))]
                for t, is_inj in order:
                    j = JG * g + t
                    sl = s_t[:, t * QC + s_g : (t + 1) * QC]
                    if is_inj:
                        nc.tensor.matmul(
                            sl, ident_sb[:],
                            bias_sb[:, boff + t * QC + s_g
                                    : boff + (t + 1) * QC],
                            start=True, stop=False,
                            skip_group_check=ro)
                    else:
                        nc.tensor.matmul(
                            sl,
                            kt_sb[p][:, j * KB : (j + 1) * KB],
                            qt_sb[p][:, qc * QC + s_g : (qc + 1) * QC],
                            start=not inj, stop=True,
                            skip_group_check=ro)
                if inj:
                    r_t = work_pool.tile([128, JG * QC], DT, tag="rhs",
                                         bufs=wb, name="r_t")
                    if s_g:
                        s3 = s_t[:].rearrange(
                            "p (t w) -> p t w", t=JG)[:, :, s_g:]
                        r3 = r_t[:].rearrange(
                            "p (t w) -> p t w", t=JG)[:, :, s_g:]
                        nc.scalar.activation(
                            r3, s3, mybir.ActivationFunctionType.Exp)
                    else:
                        nc.scalar.activation(
                            r_t[:], s_t[:], mybir.ActivationFunctionType.Exp)
                else:
                    e_t = work_pool.tile([128, JG * QC], DT, tag="exp",
                                         bufs=wb, name="e_t")
                    nc.scalar.activation(
                        e_t[:], s_t[:], mybir.ActivationFunctionType.Exp)
                    r_t = work_pool.tile([128, JG * QC], DT, tag="rhs",
                                         bufs=wb, name="r_t")
                    nc.vector.tensor_mul(
                        r_t[:], e_t[:], expb_sb[:, boff : boff + JG * QC])
                return r_t

            def compute_inj():
                # bias pre-injected into PSUM by an identity matmul:
                # chain is PE -> ACT -> PE (DVE only does output copies).
                wb = {"inj8": 8, "injst": 6}.get(VARIANT, 4)
                stb = 4 if VARIANT == "injst" else 2
                for p in range(PAIRS):
                    for qc in range(NQC):
                        gs = [g for g in range(NG)
                              if ranges[g] is not None
                              and ranges[g][0] <= qc <= ranges[g][1]]
                        if not gs:
                            continue
                        acc = acc_pool.tile([E + 1, QC], f32, tag="acc")
                        nmm = JG * len(gs)
                        mm = 0
                        for gi, g in enumerate(gs):
                            boff = goff[g] + (qc - ranges[g][0]) * JG * QC
                            s_g = 0
                            if VARIANT == "injtrim" and trim and gi > 0:
                                s_g = min(trim[JG * g + t][qc]
                                          for t in range(JG))
                                s_g = 0 if s_g >= QC else s_g
                            r_t = make_rhs(p, g, qc, boff, 3, wb, s_g)
                            for t in range(JG):
                                j = JG * g + t
                                voff = (p * NKB + j) * VW
                                nc.tensor.matmul(
                                    acc[:, s_g:],
                                    vaug_sb[:, voff : voff + E + 1],
                                    r_t[:, t * QC + s_g : (t + 1) * QC],
                                    start=(mm == 0), stop=(mm == nmm - 1))
                                mm += 1
                        st = out_pool.tile([E + 1, QC], f32, tag="st",
                                           bufs=stb, name="st")
                        nc.vector.tensor_copy(st[:], acc[:])
                        nc.sync.dma_start(
                            ot[p][:, qc * QC : (qc + 1) * QC], st[:])

            def make_rhs_pipe(p, g, qc, boff, dd, is_mul=False):
                """S-block with per-half leading-col trim dd=(d0, d1).
                Inject covers [dmin:] on both halves so exp reads only
                defined PSUM; K matmuls cover each half's own [d_t:].
                is_mul: apply the bias as exp(S)*expb on DVE instead of a
                PE inject (only used for untrimmed blocks)."""
                dmin = min(dd)
                s_t = s_pool.tile([128, JG * QC], f32, tag="s", bufs=3,
                                  name="s_t")
                if not is_mul:
                    for t in range(JG):
                        nc.tensor.matmul(
                            s_t[:, t * QC + dmin : (t + 1) * QC],
                            ident_sb[:],
                            bias_sb[:, boff + t * QC + dmin
                                    : boff + (t + 1) * QC],
                            start=True, stop=False, skip_group_check=True)
                for t in range(JG):
                    if dd[t] >= QC:
                        continue
                    j = JG * g + t
                    nc.tensor.matmul(
                        s_t[:, t * QC + dd[t] : (t + 1) * QC],
                        kt_sb[p][:, j * KB : (j + 1) * KB],
                        qt_sb[p][:, qc * QC + dd[t] : (qc + 1) * QC],
                        start=is_mul, stop=True, skip_group_check=True)
                r_t = work_pool.tile([128, JG * QC], DT, tag="rhs", bufs=8,
                                     name="r_t")
                if is_mul:
                    e_t = work_pool.tile([128, JG * QC], DT, tag="exp",
                                         bufs=4, name="e_t")
                    nc.scalar.activation(
                        e_t[:], s_t[:], mybir.ActivationFunctionType.Exp)
                    nc.vector.tensor_mul(
                        r_t[:], e_t[:], expb_sb[:, boff : boff + JG * QC])
                elif dmin:
                    s3 = s_t[:].rearrange("p (t w) -> p t w", t=JG)[:, :, dmin:]
                    r3 = r_t[:].rearrange("p (t w) -> p t w", t=JG)[:, :, dmin:]
                    nc.scalar.activation(
                        r3, s3, mybir.ActivationFunctionType.Exp)
                else:
                    nc.scalar.activation(
                        r_t[:], s_t[:], mybir.ActivationFunctionType.Exp)
                return r_t

            def compute_pipe():
                # Two software-pipelined chains (A: pairs 0/2, B: 1/3); each
                # block's AV matmuls are emitted one block AFTER its S-block,
                # with the other chain's work in between, so the in-order PE
                # never waits on ACT's exp.
                dtab = (trim if VARIANT in ("pipet", "mix")
                        and trim is not None
                        else tuple((0,) * NQC for _ in range(NKB)))
                mulset = (_mix_mul_blocks(ranges, dtab)
                          if VARIANT == "mix" else frozenset())

                def blocks_for(plist):
                    blks = []
                    for p in plist:
                        for qc in range(NQC):
                            gs = [g for g in range(NG)
                                  if ranges[g] is not None
                                  and ranges[g][0] <= qc <= ranges[g][1]]
                            nmm = sum(1 for g in gs for t in range(JG)
                                      if dtab[JG * g + t][qc] < QC)
                            for i, g in enumerate(gs):
                                boff = (goff[g]
                                        + (qc - ranges[g][0]) * JG * QC)
                                blks.append(dict(
                                    p=p, qc=qc, g=g, boff=boff,
                                    first=(i == 0), last=(i == len(gs) - 1),
                                    nmm=nmm))
                    return blks

                chains = [blocks_for([0, 2]), blocks_for([1, 3])]
                nb = len(chains[0])
                assert len(chains[1]) == nb and nb >= 2
                rstore = {}
                st_acc = [None, None]
                st_mm = [0, 0]

                def emit_S(c, k):
                    b = chains[c][k]
                    dd = tuple(dtab[JG * b["g"] + t][b["qc"]]
                               for t in range(JG))
                    rstore[(c, k)] = (
                        make_rhs_pipe(b["p"], b["g"], b["qc"], b["boff"], dd,
                                      (b["g"], b["qc"]) in mulset),
                        dd)

                def emit_AV(c, k):
                    b = chains[c][k]
                    r_t, dd = rstore.pop((c, k))
                    if st_mm[c] == 0:
                        st_acc[c] = acc_pool.tile([E + 1, QC], f32,
                                                  tag="acc", name="acc")
                    for t in range(JG):
                        if dd[t] >= QC:
                            continue
                        j = JG * b["g"] + t
                        voff = (b["p"] * NKB + j) * VW
                        nc.tensor.matmul(
                            st_acc[c][:, dd[t]:],
                            vaug_sb[:, voff : voff + E + 1],
                            r_t[:, t * QC + dd[t] : (t + 1) * QC],
                            start=(st_mm[c] == 0),
                            stop=(st_mm[c] == b["nmm"] - 1))
                        st_mm[c] += 1
                    if b["last"]:
                        st = out_pool.tile([E + 1, QC], f32, tag="st",
                                           bufs=4, name="st")
                        nc.vector.tensor_copy(st[:], st_acc[c][:])
                        nc.sync.dma_start(
                            ot[b["p"]][:, b["qc"] * QC : (b["qc"] + 1) * QC],
                            st[:])
                        st_mm[c] = 0

                # AV lags its S-block by LAG blocks so exp (and the DVE
                # multiply on mix blocks) completes before the PE reaches
                # the AV in its in-order stream.
                LAG = 2 if mulset else 1
                for k in range(LAG):
                    emit_S(0, k)
                    emit_S(1, k)
                for k in range(LAG, nb):
                    emit_S(0, k)
                    emit_AV(0, k - LAG)
                    emit_S(1, k)
                    emit_AV(1, k - LAG)
                for k in range(nb - LAG, nb):
                    emit_AV(0, k)
                    emit_AV(1, k)

            def compute_duo():
                # two independent pair-chains (A: pairs 0/1, B: pairs 2/3)
                # interleaved; keeps s_pool triple-buffered (2+3*2=8 banks).
                wb = 8 if VARIANT == "duo8" else 3
                for half in range(2):
                    pA, pB = 2 * half, 2 * half + 1
                    for qc in range(NQC):
                        gs = [g for g in range(NG)
                              if ranges[g] is not None
                              and ranges[g][0] <= qc <= ranges[g][1]]
                        if not gs:
                            continue
                        accs = {p: acc_pool.tile([E + 1, QC], f32,
                                                 tag=f"acc{p % 2}", bufs=1,
                                                 name=f"acc{p}")
                                for p in (pA, pB)}
                        nmm = JG * len(gs)
                        mms = {pA: 0, pB: 0}
                        for g in gs:
                            boff = goff[g] + (qc - ranges[g][0]) * JG * QC
                            for p in (pA, pB):
                                r_t = make_rhs(p, g, qc, boff, 3, wb)
                                for t in range(JG):
                                    j = JG * g + t
                                    voff = (p * NKB + j) * VW
                                    nc.tensor.matmul(
                                        accs[p][:],
                                        vaug_sb[:, voff : voff + E + 1],
                                        r_t[:, t * QC : (t + 1) * QC],
                                        start=(mms[p] == 0),
                                        stop=(mms[p] == nmm - 1),
                                    )
                                    mms[p] += 1
                        for p in (pA, pB):
                            st = out_pool.tile([E + 1, QC], f32, tag="st",
                                               bufs=4, name="st")
                            nc.vector.tensor_copy(st[:], accs[p][:])
                            nc.sync.dma_start(
                                ot[p][:, qc * QC : (qc + 1) * QC], st[:])

            def compute():
              if VARIANT in ("pipe", "pipet", "mix"):
                  compute_pipe()
                  return
              if VARIANT in ("inj", "inj8", "injst", "injtrim", "injro"):
                  compute_inj()
                  return
              if VARIANT in ("duo", "duo8", "injduo", "injroduo"):
                  compute_duo()
                  return
              if VARIANT == "empty":
                  st = out_pool.tile([E + 1, QC], f32, tag="st", name="st0")
                  nc.vector.tensor_copy(st[:], expb_sb[: E + 1, :QC])
                  return
              if VARIANT == "ilv":
                  compute_ilv()
                  return
              npairs = 2 if VARIANT == "half" else PAIRS
              for p in range(npairs):
                qts = qt_sb[p]
                kts = kt_sb[p]
                lo, hi = 0, 64
                for qc in range(NQC):
                    gs = [g for g in range(NG)
                          if ranges[g] is not None
                          and ranges[g][0] <= qc <= ranges[g][1]]
                    if not gs:
                        continue
                    acc = acc_pool.tile([E + 1, QC], f32, tag="acc")
                    nmm = JG * len(gs) * (2 if VARIANT == "pe2x" else 1)
                    mm = 0
                    for g in gs:
                        s_t = s_pool.tile([128, JG * QC], f32, tag="s")
                        for t in range(JG):
                            j = JG * g + t
                            for _dup in range(2 if VARIANT == "pe2x" else 1):
                                nc.tensor.matmul(
                                    s_t[:, t * QC : (t + 1) * QC],
                                    kts[lo:hi, j * KB : (j + 1) * KB],
                                    qts[lo:hi, qc * QC : (qc + 1) * QC],
                                    start=True, stop=True,
                                )
                        e_t = work_pool.tile([128, JG * QC], DT, tag="exp")
                        nc.scalar.activation(
                            e_t[:], s_t[:], mybir.ActivationFunctionType.Exp
                        )
                        if VARIANT == "act2x":
                            e_t2 = work_pool.tile([128, JG * QC], DT, tag="exp2")
                            nc.scalar.copy(e_t2[:], e_t[:])
                            e_t = e_t2
                        r_t = work_pool.tile([128, JG * QC], DT, tag="rhs")
                        boff = goff[g] + (qc - ranges[g][0]) * JG * QC
                        nc.vector.tensor_mul(
                            r_t[:], e_t[:], expb_sb[:, boff : boff + JG * QC]
                        )
                        if VARIANT == "dve2x":
                            r_t2 = work_pool.tile([128, JG * QC], DT, tag="rhs2")
                            nc.vector.tensor_mul(
                                r_t2[:], r_t[:], expb_sb[:, boff : boff + JG * QC]
                            )
                            r_t = r_t2
                        for t in range(JG):
                            j = JG * g + t
                            voff = (p * NKB + j) * VW
                            for dup in range(2 if VARIANT == "pe2x" else 1):
                                nc.tensor.matmul(
                                    acc[:],
                                    vaug_sb[:, voff : voff + E + 1],
                                    r_t[:, t * QC : (t + 1) * QC],
                                    start=(mm == 0), stop=(mm == nmm - 1),
                                )
                                mm += 1
                    st = out_pool.tile([E + 1, QC], f32, tag="st")
                    nc.vector.tensor_copy(st[:], acc[:])
                    nc.sync.dma_start(ot[p][:, qc * QC : (qc + 1) * QC], st[:])

            def compute_n():
                compute()
                if VARIANT == "dbl":
                    compute()

            if REPS > 1:
                hints = (mybir.EngineType.PE, mybir.EngineType.Activation,
                         mybir.EngineType.DVE, mybir.EngineType.SP)
                with tc.For_i(0, REPS, 1, hint_engines=hints):
                    compute_n()
            else:
                compute_n()
    nc.compile()
    return nc


MASK_NEG = -30000.0  # additive bias for masked entries (fp16-safe, exp -> 0)


def _pack(matT, ranges, altT=None, altset=frozenset()):
    """Pack a [k, q]-oriented (L, L) matrix into the group-major SBUF layout:
    per group g, per qc in its range, [j0-chunk | j1-chunk]. Blocks in
    altset take their content from altT instead (mix variant)."""
    chunks = []
    for g, r in enumerate(ranges):
        if r is None:
            continue
        for qc in range(r[0], r[1] + 1):
            src = altT if (g, qc) in altset else matT
            for t in range(JG):
                j = JG * g + t
                chunks.append(
                    src[j * KB : (j + 1) * KB, qc * QC : (qc + 1) * QC])
    return np.ascontiguousarray(np.concatenate(chunks, axis=1))


def prepare(queries, keys, values, attn_mask, causal_mask):
    """Host-side prep: block plan + per-core input maps."""
    queries = np.asarray(queries, dtype=np.float32)
    keys = np.asarray(keys, dtype=np.float32)
    values = np.asarray(values, dtype=np.float32)
    attn_mask = np.asarray(attn_mask).astype(bool).reshape(L, L)
    causal_mask = np.asarray(causal_mask, dtype=np.float32)
    assert queries.shape == (B, L, H, E)

    # exp of the scaled additive bias, 0 where masked; [k, q] orientation
    expbT = np.where(attn_mask, 0.0, np.exp(SCALE * causal_mask)).T
    expbT = np.ascontiguousarray(expbT, dtype=NPDT)
    # raw additive bias with masked -> large negative (inject path)
    biasT = np.where(attn_mask, MASK_NEG, SCALE * causal_mask).T
    biasT = np.ascontiguousarray(biasT, dtype=NPDT)

    ranges = _plan(expbT != 0)
    trim = _plan_trim(expbT != 0)
    expb_packed = _pack(expbT, ranges)
    if VARIANT == "mix":
        # mul-blocks carry expb (the DVE multiplicand); the rest carry the
        # raw bias for the PE inject. Same packed layout either way.
        mulset = _mix_mul_blocks(ranges, trim)
        bias_packed = _pack(biasT, ranges, expbT, mulset)
    else:
        bias_packed = _pack(biasT, ranges)
    ident = np.eye(128, dtype=NPDT)

    in_maps = []
    for c in range(NCORES):
        qts, kts = [], []
        va = np.zeros((128, PAIRS * NKB * VW), dtype=NPDT)
        for p in range(PAIRS):
            qts.append((queries[p, :, c, :].T * SCALE).astype(NPDT))
            kts.append(keys[p, :, c, :].T.astype(NPDT))
            vp = values[p, :, c, :].astype(NPDT)  # (L, 64)
            for j in range(NKB):
                col = (p * NKB + j) * VW
                va[:, col : col + E] = vp[j * KB : (j + 1) * KB, :]
                va[:, col + E] = 1.0
        im = {"vaug": va, "expb": expb_packed, "biast": bias_packed,
              "ident": ident}
        for p in range(PAIRS):
            im[f"qt{p}"] = np.ascontiguousarray(qts[p])
            im[f"kt{p}"] = np.ascontiguousarray(kts[p])
        in_maps.append(im)
    return ranges, trim, in_maps


def kernel(queries, keys, values, attn_mask, causal_mask):
    ranges, trim, in_maps = prepare(queries, keys, values, attn_mask,
                                    causal_mask)
    key = (ranges, trim, VARIANT, MIX_MUL)
    if key not in _cache:
        _cache[key] = _build(ranges, trim)
    nc = _cache[key]

    res = run_bass_kernel_spmd(nc, in_maps, list(range(NCORES)), trace=TRACE)
    LAST["results"] = res

    out = np.empty((B, L, H, E), dtype=np.float32)
    for c in range(NCORES):
        ot = res.results[c]["ot"]  # (PAIRS, 65, L)
        for p in range(PAIRS):
            out[p, :, c, :] = (ot[p, :E, :] / ot[p, E : E + 1, :]).T
    return out

